# revision 16
# baseline (speedup 1.0000x reference)
"""Adaptive Computation Time kernel for 8 TRN2 NeuronCores.

Reference semantics: p = sigmoid(x @ W + b) is constant across the 20 ACT
steps, so the scan has a closed form per position:
  N_raw = floor(0.99/p) + 1          (first step n with n*p > 0.99)
  halted = N_raw <= 20
  e = min(floor(0.99/p), 20)          (exponent; = N-1 if halted else 20)
  state  = x * (1 - (1-p)^e * (halted ? e*p : 1))
  ponder = halted ? (e+1) + 1 - (e+1)*p : 20
with (1-p)^e = exp(e * ln(1-p)).

Sharding: data-parallel over batch (B=8 -> 1 batch element per core).

Engine plan per core (32 row-tiles of (128,1024)):
  sync seq   : 32 x-tile loads + const loads      (DMA, ~47us traffic)
  DVE        : 32 affine_mul_reduce (z=x.W) + half the scale-muls + reciprocal
  ACT        : Exp/Ln (one act table) + other half of scale-muls
  GpSimd     : closed-form elementwise chains on (128,G) z-groups
  tensor seq : 32 state-tile stores               (DMA, ~47us traffic)
"""

import os
import sys

import numpy as np


def _ensure_paths():
    for p in (
        "/root/.axon_site",
        "/root/.axon_site/_ro/trn_rl_repo",
        "/root/.axon_site/_ro/pypackages",
        "/opt/trn_rl_repo",
        "/opt/pypackages",
    ):
        if os.path.isdir(p) and p not in sys.path:
            sys.path.append(p)


_ensure_paths()

import concourse.bass as bass  # noqa: E402
import concourse.bacc as bacc  # noqa: E402
import concourse.mybir as mybir  # noqa: E402
import concourse.tile as tile  # noqa: E402
import concourse.hw_specs as hw_specs  # noqa: E402
from concourse.bass_utils import run_bass_kernel_spmd  # noqa: E402

B, S, D = 8, 4096, 1024
N_CORES = 8
ROWS = (B * S) // N_CORES  # 4096 rows per core
P = 128
NT = ROWS // P  # 32 row-tiles per core
G = 8  # tiles per closed-form group
THRESH = float(np.float32(0.99))
F32 = mybir.dt.float32
A = mybir.AluOpType
AF = mybir.ActivationFunctionType

_ACT_TABLE = "natural_log_exp_and_others"


def _patch_act_tables():
    """Force the act-table pass to use only the table holding Exp+Ln+Copy.

    The greedy pass otherwise bounces between an Exp-only and an Ln-only
    table (1.28us per load, 5 loads).  Positions must be preserved because
    the list index is the act_func_set_id walrus remaps.
    """
    if getattr(bacc, "_act_tables_patched", False):
        return

    def _filtered(arch):
        t = hw_specs.get_activation_tables(arch)
        return {n: (f if n == _ACT_TABLE else set()) for n, f in t.items()}

    bacc.get_activation_tables = _filtered
    bacc._act_tables_patched = True


def build_nc():
    _patch_act_tables()
    nc = bacc.Bacc("TRN2", target_bir_lowering=False, debug=False)

    x_ext = nc.declare_dram_parameter("x", [ROWS, D], F32, isOutput=False)
    w_ext = nc.declare_dram_parameter("W", [P, D], F32, isOutput=False)  # replicated
    b_ext = nc.declare_dram_parameter("nb", [P, 1], F32, isOutput=False)  # -b replicated
    st_ext = nc.declare_dram_parameter("out_state", [ROWS, D], F32, isOutput=True)
    pd_ext = nc.declare_dram_parameter("out_ponder", [ROWS, 1], F32, isOutput=True)

    # ponder viewed as (p, t): row 128*t + p  ->  partition p, column t
    pd_view = pd_ext[:, :].rearrange("(t p) one -> p (t one)", p=P)

    with tile.TileContext(nc) as tc:
        with (
            tc.tile_pool(name="const", bufs=1) as cpool,
            tc.tile_pool(name="xdata", bufs=NT) as xpool,
            tc.tile_pool(name="scratch", bufs=2) as spool,
            tc.tile_pool(name="small", bufs=2) as smp,
        ):
            # --- W and -b arrive pre-replicated across partitions ---
            wb = cpool.tile([P, D], F32)
            nc.sync.dma_start(wb[:], w_ext[:, :])
            nbcol = cpool.tile([P, 1], F32)
            nc.sync.dma_start(nbcol[:], b_ext[:, :])

            for g in range(NT // G):
                zb = smp.tile([P, G], F32, tag="zb")
                xts = []
                for j in range(G):
                    t = g * G + j
                    xt = xpool.tile([P, D], F32, tag="xt")
                    nc.sync.dma_start(xt[:], x_ext[bass.ts(t, P), :])
                    xts.append(xt)
                    scr = spool.tile([P, D], F32, tag="scr")
                    nc.vector.affine_mul_reduce(
                        out=scr[:],
                        accum_out=zb[:, j : j + 1],
                        in0=xt[:],
                        in1=wb[:],
                        scale=1.0,
                        bias=0.0,
                    )

                # --- closed form on zb (P, G); elementwise on GpSimd ---
                def st(tag):
                    return smp.tile([P, G], F32, tag=tag, name=tag)

                ez = st("ez")
                nc.scalar.activation(
                    ez[:], zb[:], AF.Exp, scale=-1.0, bias=nbcol[:]
                )  # e^-(z+b)
                den = st("den")
                nc.vector.tensor_scalar(den[:], ez[:], 1e37, 1.0, A.min, A.add)  # 1/p
                p_ = st("p")
                nc.vector.reciprocal(p_[:], den[:])
                om = st("om")
                nc.vector.tensor_scalar(om[:], p_[:], -1.0, 1.0, A.mult, A.add)  # 1-p
                om2 = st("om2")
                nc.vector.tensor_scalar(om2[:], om[:], 1e-38, None, A.max)
                lom = st("lom")
                nc.scalar.activation(lom[:], om2[:], AF.Ln)  # ln(1-p)
                q = st("q")
                nc.vector.tensor_scalar(q[:], den[:], THRESH, 30.0, A.mult, A.min)
                # e2 = min(floor(q), 20): cast-based floor with +-1 correction,
                # correct under any f32->i32 rounding mode (HW rounds, sim truncs)
                q2 = st("q2")
                nc.vector.tensor_scalar(q2[:], q[:], 0.5, None, A.subtract)
                qi = smp.tile([P, G], mybir.dt.int32, tag="qi", name="qi")
                nc.vector.tensor_copy(qi[:], q2[:])
                qf = st("qf")
                nc.vector.tensor_copy(qf[:], qi[:])
                c1 = st("c1")
                nc.vector.scalar_tensor_tensor(
                    c1[:], q[:], 1.0, qf[:], A.subtract, A.is_ge
                )  # [q-1 >= qf] = [q >= qf+1]
                c2 = st("c2")
                nc.vector.tensor_tensor(c2[:], q[:], qf[:], A.is_lt)
                ra = st("ra")
                nc.vector.tensor_tensor(ra[:], qf[:], c1[:], A.add)
                rb = st("rb")
                nc.vector.tensor_tensor(rb[:], ra[:], c2[:], A.subtract)
                e2 = st("e2")
                nc.vector.tensor_scalar(e2[:], rb[:], 20.0, None, A.min)
                h = st("h")
                nc.vector.tensor_scalar(h[:], e2[:], 19.5, None, A.is_lt)
                esp = st("esp")
                nc.vector.tensor_tensor(esp[:], e2[:], lom[:], A.mult)
                pw = st("pw")
                nc.scalar.activation(pw[:], esp[:], AF.Exp)  # (1-p)^e
                ep = st("ep")
                nc.vector.tensor_tensor(ep[:], e2[:], p_[:], A.mult)
                v = st("v")
                nc.vector.scalar_tensor_tensor(
                    v[:], ep[:], 1.0, h[:], A.subtract, A.mult
                )  # (ep-1)*h = u-h
                w2 = st("w2")
                nc.vector.tensor_tensor(w2[:], v[:], pw[:], A.mult)
                mm = st("mm")
                nc.vector.tensor_tensor(mm[:], pw[:], w2[:], A.add)
                mb = st("mb")
                nc.vector.tensor_scalar(mb[:], mm[:], -1.0, 1.0, A.mult, A.add)

                a2 = st("a2")
                nc.vector.tensor_tensor(a2[:], e2[:], ep[:], A.subtract)
                pp = st("pp")
                nc.vector.tensor_scalar(pp[:], p_[:], 18.0, None, A.add)
                b3 = st("b3")
                nc.vector.tensor_tensor(b3[:], pp[:], a2[:], A.subtract)
                c3 = st("c3")
                nc.vector.tensor_tensor(c3[:], h[:], b3[:], A.mult)
                pd = st("pd")
                nc.vector.tensor_scalar(pd[:], c3[:], -1.0, 20.0, A.mult, A.add)

                # --- scale x by m in place (alternate ACT/DVE), store out ---
                for j in range(G):
                    t = g * G + j
                    mcol = mb[:, j : j + 1]
                    if j % 2 == 0:
                        nc.scalar.activation(xts[j][:], xts[j][:], AF.Copy, scale=mcol)
                        nc.scalar.dma_start(st_ext[bass.ts(t, P), :], xts[j][:])
                    else:
                        nc.vector.tensor_scalar(
                            xts[j][:], xts[j][:], mcol, None, A.mult
                        )
                        nc.gpsimd.dma_start(st_ext[bass.ts(t, P), :], xts[j][:])

                nc.gpsimd.dma_start(pd_view[:, g * G : (g + 1) * G], pd[:])

    nc.compile()
    return nc


_NC_CACHE = []


def _get_nc():
    if not _NC_CACHE:
        _NC_CACHE.append(build_nc())
    return _NC_CACHE[0]


def make_in_maps(x, W, b):
    x = np.ascontiguousarray(x, dtype=np.float32)
    Wt = np.ascontiguousarray(
        np.broadcast_to(W.reshape(1, D), (P, D)), dtype=np.float32
    )
    nb = np.full((P, 1), -float(np.asarray(b).reshape(-1)[0]), dtype=np.float32)
    xs = x.reshape(N_CORES, ROWS, D)
    return [{"x": xs[c], "W": Wt, "nb": nb} for c in range(N_CORES)]


def kernel(x: np.ndarray, W: np.ndarray, b: np.ndarray):
    nc = _get_nc()
    in_maps = make_in_maps(x, W, b)
    res = run_bass_kernel_spmd(nc, in_maps, core_ids=list(range(N_CORES)))
    state = np.stack(
        [res.results[c]["out_state"] for c in range(N_CORES)], axis=0
    ).reshape(B, S, D)
    ponder = np.stack(
        [res.results[c]["out_ponder"] for c in range(N_CORES)], axis=0
    ).reshape(B, S, 1)
    return state, ponder


# revision 17
# speedup vs baseline: 1.0981x; 1.0981x over previous
"""Adaptive Computation Time kernel for 8 TRN2 NeuronCores.

Reference semantics: p = sigmoid(x @ W + b) is constant across the 20 ACT
steps, so the scan has a closed form per position:
  N_raw = floor(0.99/p) + 1          (first step n with n*p > 0.99)
  halted = N_raw <= 20
  e = min(floor(0.99/p), 20)          (exponent; = N-1 if halted else 20)
  state  = x * (1 - (1-p)^e * (halted ? e*p : 1))
  ponder = halted ? (e+1) + 1 - (e+1)*p : 20
with (1-p)^e = exp(e * ln(1-p)).

Sharding: data-parallel over batch (B=8 -> 1 batch element per core).

Engine plan per core (32 row-tiles of (128,1024)):
  sync seq   : 32 x-tile loads + const loads      (DMA, ~47us traffic)
  DVE        : 32 affine_mul_reduce (z=x.W) + half the scale-muls + reciprocal
  ACT        : Exp/Ln (one act table) + other half of scale-muls
  GpSimd     : closed-form elementwise chains on (128,G) z-groups
  tensor seq : 32 state-tile stores               (DMA, ~47us traffic)
"""

import os
import sys

import numpy as np


def _ensure_paths():
    for p in (
        "/root/.axon_site",
        "/root/.axon_site/_ro/trn_rl_repo",
        "/root/.axon_site/_ro/pypackages",
        "/opt/trn_rl_repo",
        "/opt/pypackages",
    ):
        if os.path.isdir(p) and p not in sys.path:
            sys.path.append(p)


_ensure_paths()

import concourse.bass as bass  # noqa: E402
import concourse.bacc as bacc  # noqa: E402
import concourse.mybir as mybir  # noqa: E402
import concourse.tile as tile  # noqa: E402
import concourse.hw_specs as hw_specs  # noqa: E402
from concourse.bass_utils import run_bass_kernel_spmd  # noqa: E402

B, S, D = 8, 4096, 1024
N_CORES = 8
ROWS = (B * S) // N_CORES  # 4096 rows per core
P = 128
NT = ROWS // P  # 32 row-tiles per core
G = 8  # tiles per closed-form group
THRESH = float(np.float32(0.99))
F32 = mybir.dt.float32
A = mybir.AluOpType
AF = mybir.ActivationFunctionType

_ACT_TABLE = "natural_log_exp_and_others"


def _patch_act_tables():
    """Force the act-table pass to use only the table holding Exp+Ln+Copy.

    The greedy pass otherwise bounces between an Exp-only and an Ln-only
    table (1.28us per load, 5 loads).  Positions must be preserved because
    the list index is the act_func_set_id walrus remaps.
    """
    if getattr(bacc, "_act_tables_patched", False):
        return

    def _filtered(arch):
        t = hw_specs.get_activation_tables(arch)
        return {n: (f if n == _ACT_TABLE else set()) for n, f in t.items()}

    bacc.get_activation_tables = _filtered
    bacc._act_tables_patched = True


def build_nc():
    _patch_act_tables()
    nc = bacc.Bacc("TRN2", target_bir_lowering=False, debug=False)

    x_ext = nc.declare_dram_parameter("x", [ROWS, D], F32, isOutput=False)
    w_ext = nc.declare_dram_parameter("W", [P, D], F32, isOutput=False)  # replicated
    b_ext = nc.declare_dram_parameter("nb", [P, 1], F32, isOutput=False)  # -b replicated
    st_ext = nc.declare_dram_parameter("out_state", [ROWS, D], F32, isOutput=True)
    pd_ext = nc.declare_dram_parameter("out_ponder", [ROWS, 1], F32, isOutput=True)

    # ponder viewed as (p, t): row 128*t + p  ->  partition p, column t
    pd_view = pd_ext[:, :].rearrange("(t p) one -> p (t one)", p=P)

    with tile.TileContext(nc) as tc:
        with (
            tc.tile_pool(name="const", bufs=1) as cpool,
            tc.tile_pool(name="xdata", bufs=NT) as xpool,
            tc.tile_pool(name="scratch", bufs=2) as spool,
            tc.tile_pool(name="small", bufs=4) as smp,
        ):
            # --- W and -b arrive pre-replicated across partitions ---
            wb = cpool.tile([P, D], F32)
            nc.sync.dma_start(wb[:], w_ext[:, :])
            nbcol = cpool.tile([P, 1], F32)
            nc.sync.dma_start(nbcol[:], b_ext[:, :])

            for g in range(NT // G):
                zb = smp.tile([P, G], F32, tag="zb")
                xts = []
                for j in range(G):
                    t = g * G + j
                    xt = xpool.tile([P, D], F32, tag="xt")
                    nc.sync.dma_start(xt[:], x_ext[bass.ts(t, P), :])
                    xts.append(xt)
                    scr = spool.tile([P, D], F32, tag="scr")
                    nc.vector.affine_mul_reduce(
                        out=scr[:],
                        accum_out=zb[:, j : j + 1],
                        in0=xt[:],
                        in1=wb[:],
                        scale=1.0,
                        bias=0.0,
                    )

                # --- closed form on zb (P, G); elementwise on GpSimd ---
                def st(tag):
                    return smp.tile([P, G], F32, tag=tag, name=tag)

                ez = st("ez")
                nc.scalar.activation(
                    ez[:], zb[:], AF.Exp, scale=-1.0, bias=nbcol[:]
                )  # e^-(z+b)
                den = st("den")
                nc.vector.tensor_scalar(den[:], ez[:], 1e37, 1.0, A.min, A.add)  # 1/p
                p_ = st("p")
                nc.vector.reciprocal(p_[:], den[:])
                om = st("om")
                nc.vector.tensor_scalar(om[:], p_[:], -1.0, 1.0, A.mult, A.add)  # 1-p
                om2 = st("om2")
                nc.vector.tensor_scalar(om2[:], om[:], 1e-38, None, A.max)
                lom = st("lom")
                nc.scalar.activation(lom[:], om2[:], AF.Ln)  # ln(1-p)
                q = st("q")
                nc.vector.tensor_scalar(q[:], den[:], THRESH, 30.0, A.mult, A.min)
                # e2 = min(floor(q), 20): cast-based floor with +-1 correction,
                # correct under any f32->i32 rounding mode (HW rounds, sim truncs)
                q2 = st("q2")
                nc.vector.tensor_scalar(q2[:], q[:], 0.5, None, A.subtract)
                qi = smp.tile([P, G], mybir.dt.int32, tag="qi", name="qi")
                nc.vector.tensor_copy(qi[:], q2[:])
                qf = st("qf")
                nc.vector.tensor_copy(qf[:], qi[:])
                c1 = st("c1")
                nc.vector.scalar_tensor_tensor(
                    c1[:], q[:], 1.0, qf[:], A.subtract, A.is_ge
                )  # [q-1 >= qf] = [q >= qf+1]
                c2 = st("c2")
                nc.vector.tensor_tensor(c2[:], q[:], qf[:], A.is_lt)
                ra = st("ra")
                nc.vector.tensor_tensor(ra[:], qf[:], c1[:], A.add)
                rb = st("rb")
                nc.vector.tensor_tensor(rb[:], ra[:], c2[:], A.subtract)
                e2 = st("e2")
                nc.vector.tensor_scalar(e2[:], rb[:], 20.0, None, A.min)
                h = st("h")
                nc.vector.tensor_scalar(h[:], e2[:], 19.5, None, A.is_lt)
                esp = st("esp")
                nc.vector.tensor_tensor(esp[:], e2[:], lom[:], A.mult)
                pw = st("pw")
                nc.scalar.activation(pw[:], esp[:], AF.Exp)  # (1-p)^e
                ep = st("ep")
                nc.vector.tensor_tensor(ep[:], e2[:], p_[:], A.mult)
                v = st("v")
                nc.vector.scalar_tensor_tensor(
                    v[:], ep[:], 1.0, h[:], A.subtract, A.mult
                )  # (ep-1)*h = u-h
                w2 = st("w2")
                nc.vector.tensor_tensor(w2[:], v[:], pw[:], A.mult)
                mm = st("mm")
                nc.vector.tensor_tensor(mm[:], pw[:], w2[:], A.add)
                mb = st("mb")
                nc.vector.tensor_scalar(mb[:], mm[:], -1.0, 1.0, A.mult, A.add)

                a2 = st("a2")
                nc.vector.tensor_tensor(a2[:], e2[:], ep[:], A.subtract)
                pp = st("pp")
                nc.vector.tensor_scalar(pp[:], p_[:], 18.0, None, A.add)
                b3 = st("b3")
                nc.vector.tensor_tensor(b3[:], pp[:], a2[:], A.subtract)
                c3 = st("c3")
                nc.vector.tensor_tensor(c3[:], h[:], b3[:], A.mult)
                pd = st("pd")
                nc.vector.tensor_scalar(pd[:], c3[:], -1.0, 20.0, A.mult, A.add)

                # --- scale x by m in place (alternate ACT/DVE), store out ---
                for j in range(G):
                    t = g * G + j
                    mcol = mb[:, j : j + 1]
                    if j % 2 == 0:
                        nc.scalar.activation(xts[j][:], xts[j][:], AF.Copy, scale=mcol)
                    else:
                        nc.vector.tensor_scalar(
                            xts[j][:], xts[j][:], mcol, None, A.mult
                        )
                    nc.sync.dma_start(st_ext[bass.ts(t, P), :], xts[j][:])

                nc.sync.dma_start(pd_view[:, g * G : (g + 1) * G], pd[:])

    nc.compile()
    return nc


_NC_CACHE = []


def _get_nc():
    if not _NC_CACHE:
        _NC_CACHE.append(build_nc())
    return _NC_CACHE[0]


def make_in_maps(x, W, b):
    x = np.ascontiguousarray(x, dtype=np.float32)
    Wt = np.ascontiguousarray(
        np.broadcast_to(W.reshape(1, D), (P, D)), dtype=np.float32
    )
    nb = np.full((P, 1), -float(np.asarray(b).reshape(-1)[0]), dtype=np.float32)
    xs = x.reshape(N_CORES, ROWS, D)
    return [{"x": xs[c], "W": Wt, "nb": nb} for c in range(N_CORES)]


def kernel(x: np.ndarray, W: np.ndarray, b: np.ndarray):
    nc = _get_nc()
    in_maps = make_in_maps(x, W, b)
    res = run_bass_kernel_spmd(nc, in_maps, core_ids=list(range(N_CORES)))
    state = np.stack(
        [res.results[c]["out_state"] for c in range(N_CORES)], axis=0
    ).reshape(B, S, D)
    ponder = np.stack(
        [res.results[c]["out_ponder"] for c in range(N_CORES)], axis=0
    ).reshape(B, S, 1)
    return state, ponder


# revision 18
# speedup vs baseline: 1.3030x; 1.1866x over previous
"""Adaptive Computation Time kernel for 8 TRN2 NeuronCores.

Reference semantics: p = sigmoid(x @ W + b) is constant across the 20 ACT
steps, so the scan has a closed form per position:
  N_raw = floor(0.99/p) + 1          (first step n with n*p > 0.99)
  halted = N_raw <= 20
  e = min(floor(0.99/p), 20)          (exponent; = N-1 if halted else 20)
  state  = x * (1 - (1-p)^e * (halted ? e*p : 1))
  ponder = halted ? (e+1) + 1 - (e+1)*p : 20
with (1-p)^e = exp(e * ln(1-p)).

Sharding: data-parallel over batch (B=8 -> 1 batch element per core).

Engine plan per core (32 row-tiles of (128,1024)):
  sync seq   : 32 x-tile loads + const loads      (DMA, ~47us traffic)
  DVE        : 32 affine_mul_reduce (z=x.W) + half the scale-muls + reciprocal
  ACT        : Exp/Ln (one act table) + other half of scale-muls
  GpSimd     : closed-form elementwise chains on (128,G) z-groups
  tensor seq : 32 state-tile stores               (DMA, ~47us traffic)
"""

import os
import sys

import numpy as np


def _ensure_paths():
    for p in (
        "/root/.axon_site",
        "/root/.axon_site/_ro/trn_rl_repo",
        "/root/.axon_site/_ro/pypackages",
        "/opt/trn_rl_repo",
        "/opt/pypackages",
    ):
        if os.path.isdir(p) and p not in sys.path:
            sys.path.append(p)


_ensure_paths()

import concourse.bass as bass  # noqa: E402
import concourse.bacc as bacc  # noqa: E402
import concourse.mybir as mybir  # noqa: E402
import concourse.tile as tile  # noqa: E402
import concourse.hw_specs as hw_specs  # noqa: E402
from concourse.bass_utils import run_bass_kernel_spmd  # noqa: E402

B, S, D = 8, 4096, 1024
N_CORES = 8
ROWS = (B * S) // N_CORES  # 4096 rows per core
P = 128
NT = ROWS // P  # 32 row-tiles per core
G = 8  # tiles per closed-form group
THRESH = float(np.float32(0.99))
F32 = mybir.dt.float32
A = mybir.AluOpType
AF = mybir.ActivationFunctionType

_ACT_TABLE = "natural_log_exp_and_others"


def _patch_act_tables():
    """Force the act-table pass to use only the table holding Exp+Ln+Copy.

    The greedy pass otherwise bounces between an Exp-only and an Ln-only
    table (1.28us per load, 5 loads).  Positions must be preserved because
    the list index is the act_func_set_id walrus remaps.
    """
    if getattr(bacc, "_act_tables_patched", False):
        return

    def _filtered(arch):
        t = hw_specs.get_activation_tables(arch)
        return {n: (f if n == _ACT_TABLE else set()) for n, f in t.items()}

    bacc.get_activation_tables = _filtered
    bacc._act_tables_patched = True


def build_nc():
    _patch_act_tables()
    nc = bacc.Bacc("TRN2", target_bir_lowering=False, debug=False)

    x_ext = nc.declare_dram_parameter("x", [ROWS, D], F32, isOutput=False)
    w_ext = nc.declare_dram_parameter("W", [P, D], F32, isOutput=False)  # replicated
    b_ext = nc.declare_dram_parameter("nb", [P, 1], F32, isOutput=False)  # -b replicated
    st_ext = nc.declare_dram_parameter("out_state", [ROWS, D], F32, isOutput=True)
    pd_ext = nc.declare_dram_parameter("out_ponder", [ROWS, 1], F32, isOutput=True)

    # ponder viewed as (p, t): row 128*t + p  ->  partition p, column t
    pd_view = pd_ext[:, :].rearrange("(t p) one -> p (t one)", p=P)

    with tile.TileContext(nc) as tc:
        with (
            tc.tile_pool(name="const", bufs=1) as cpool,
            tc.tile_pool(name="xdata", bufs=NT) as xpool,
            tc.tile_pool(name="scratch", bufs=2) as spool,
            tc.tile_pool(name="small", bufs=4) as smp,
        ):
            # --- W and -b arrive pre-replicated across partitions ---
            wb = cpool.tile([P, D], F32)
            nc.sync.dma_start(wb[:], w_ext[:, :])
            nbcol = cpool.tile([P, 1], F32)
            nc.sync.dma_start(nbcol[:], b_ext[:, :])
            pond_all = cpool.tile([P, NT], F32)

            for g in range(NT // G):
                zb = smp.tile([P, G], F32, tag="zb")
                xts = []
                for j in range(G):
                    t = g * G + j
                    xt = xpool.tile([P, D], F32, tag="xt")
                    nc.sync.dma_start(xt[:], x_ext[bass.ts(t, P), :])
                    xts.append(xt)
                    scr = spool.tile([P, D], F32, tag="scr")
                    nc.vector.affine_mul_reduce(
                        out=scr[:],
                        accum_out=zb[:, j : j + 1],
                        in0=xt[:],
                        in1=wb[:],
                        scale=1.0,
                        bias=0.0,
                    )

                # --- closed form on zb (P, G); elementwise on GpSimd ---
                def st(tag):
                    return smp.tile([P, G], F32, tag=tag, name=tag)

                ez = st("ez")
                nc.scalar.activation(
                    ez[:], zb[:], AF.Exp, scale=-1.0, bias=nbcol[:]
                )  # e^-(z+b)
                den = st("den")
                nc.vector.tensor_scalar(den[:], ez[:], 1e37, 1.0, A.min, A.add)  # 1/p
                p_ = st("p")
                nc.vector.reciprocal(p_[:], den[:])
                om = st("om")
                nc.vector.tensor_scalar(om[:], p_[:], -1.0, 1.0, A.mult, A.add)  # 1-p
                om2 = st("om2")
                nc.vector.tensor_scalar(om2[:], om[:], 1e-38, None, A.max)
                lom = st("lom")
                nc.scalar.activation(lom[:], om2[:], AF.Ln)  # ln(1-p)
                q = st("q")
                nc.vector.tensor_scalar(q[:], den[:], THRESH, 30.0, A.mult, A.min)
                # e2 = min(floor(q), 20): cast-based floor with +-1 correction,
                # correct under any f32->i32 rounding mode (HW rounds, sim truncs)
                q2 = st("q2")
                nc.vector.tensor_scalar(q2[:], q[:], 0.5, None, A.subtract)
                qi = smp.tile([P, G], mybir.dt.int32, tag="qi", name="qi")
                nc.vector.tensor_copy(qi[:], q2[:])
                qf = st("qf")
                nc.vector.tensor_copy(qf[:], qi[:])
                c1 = st("c1")
                nc.vector.scalar_tensor_tensor(
                    c1[:], q[:], 1.0, qf[:], A.subtract, A.is_ge
                )  # [q-1 >= qf] = [q >= qf+1]
                c2 = st("c2")
                nc.vector.tensor_tensor(c2[:], q[:], qf[:], A.is_lt)
                ra = st("ra")
                nc.vector.tensor_tensor(ra[:], qf[:], c1[:], A.add)
                rb = st("rb")
                nc.vector.tensor_tensor(rb[:], ra[:], c2[:], A.subtract)
                e2 = st("e2")
                nc.vector.tensor_scalar(e2[:], rb[:], 20.0, None, A.min)
                h = st("h")
                nc.vector.tensor_scalar(h[:], e2[:], 19.5, None, A.is_lt)
                esp = st("esp")
                nc.vector.tensor_tensor(esp[:], e2[:], lom[:], A.mult)
                pw = st("pw")
                nc.scalar.activation(pw[:], esp[:], AF.Exp)  # (1-p)^e
                ep = st("ep")
                nc.vector.tensor_tensor(ep[:], e2[:], p_[:], A.mult)
                v = st("v")
                nc.vector.scalar_tensor_tensor(
                    v[:], ep[:], 1.0, h[:], A.subtract, A.mult
                )  # (ep-1)*h = u-h
                w2 = st("w2")
                nc.vector.tensor_tensor(w2[:], v[:], pw[:], A.mult)
                mm = st("mm")
                nc.vector.tensor_tensor(mm[:], pw[:], w2[:], A.add)
                mb = st("mb")
                nc.vector.tensor_scalar(mb[:], mm[:], -1.0, 1.0, A.mult, A.add)

                a2 = st("a2")
                nc.vector.tensor_tensor(a2[:], e2[:], ep[:], A.subtract)
                pp = st("pp")
                nc.vector.tensor_scalar(pp[:], p_[:], 18.0, None, A.add)
                b3 = st("b3")
                nc.vector.tensor_tensor(b3[:], pp[:], a2[:], A.subtract)
                c3 = st("c3")
                nc.vector.tensor_tensor(c3[:], h[:], b3[:], A.mult)
                nc.vector.tensor_scalar(
                    pond_all[:, g * G : (g + 1) * G], c3[:], -1.0, 20.0, A.mult, A.add
                )

                # --- scale x by m in place (alternate ACT/DVE), store out ---
                for j in range(G):
                    t = g * G + j
                    mcol = mb[:, j : j + 1]
                    if j % 2 == 0:
                        nc.scalar.activation(xts[j][:], xts[j][:], AF.Copy, scale=mcol)
                    else:
                        nc.vector.tensor_scalar(
                            xts[j][:], xts[j][:], mcol, None, A.mult
                        )
                    nc.sync.dma_start(st_ext[bass.ts(t, P), :], xts[j][:])


            nc.sync.dma_start(pd_view[:, :], pond_all[:])

    nc.compile()
    return nc


_NC_CACHE = []


def _get_nc():
    if not _NC_CACHE:
        _NC_CACHE.append(build_nc())
    return _NC_CACHE[0]


def make_in_maps(x, W, b):
    x = np.ascontiguousarray(x, dtype=np.float32)
    Wt = np.ascontiguousarray(
        np.broadcast_to(W.reshape(1, D), (P, D)), dtype=np.float32
    )
    nb = np.full((P, 1), -float(np.asarray(b).reshape(-1)[0]), dtype=np.float32)
    xs = x.reshape(N_CORES, ROWS, D)
    return [{"x": xs[c], "W": Wt, "nb": nb} for c in range(N_CORES)]


def kernel(x: np.ndarray, W: np.ndarray, b: np.ndarray):
    nc = _get_nc()
    in_maps = make_in_maps(x, W, b)
    res = run_bass_kernel_spmd(nc, in_maps, core_ids=list(range(N_CORES)))
    state = np.stack(
        [res.results[c]["out_state"] for c in range(N_CORES)], axis=0
    ).reshape(B, S, D)
    ponder = np.stack(
        [res.results[c]["out_ponder"] for c in range(N_CORES)], axis=0
    ).reshape(B, S, 1)
    return state, ponder


# revision 19
# speedup vs baseline: 1.3228x; 1.0152x over previous
"""Adaptive Computation Time kernel for 8 TRN2 NeuronCores.

Reference semantics: p = sigmoid(x @ W + b) is constant across the 20 ACT
steps, so the scan has a closed form per position:
  N_raw = floor(0.99/p) + 1          (first step n with n*p > 0.99)
  halted = N_raw <= 20
  e = min(floor(0.99/p), 20)          (exponent; = N-1 if halted else 20)
  state  = x * (1 - (1-p)^e * (halted ? e*p : 1))
  ponder = halted ? (e+1) + 1 - (e+1)*p : 20
with (1-p)^e = exp(e * ln(1-p)).

Sharding: data-parallel over batch (B=8 -> 1 batch element per core).

Engine plan per core (32 row-tiles of (128,1024)):
  sync seq   : 32 x-tile loads + const loads      (DMA, ~47us traffic)
  DVE        : 32 affine_mul_reduce (z=x.W) + half the scale-muls + reciprocal
  ACT        : Exp/Ln (one act table) + other half of scale-muls
  GpSimd     : closed-form elementwise chains on (128,G) z-groups
  tensor seq : 32 state-tile stores               (DMA, ~47us traffic)
"""

import os
import sys

import numpy as np


def _ensure_paths():
    for p in (
        "/root/.axon_site",
        "/root/.axon_site/_ro/trn_rl_repo",
        "/root/.axon_site/_ro/pypackages",
        "/opt/trn_rl_repo",
        "/opt/pypackages",
    ):
        if os.path.isdir(p) and p not in sys.path:
            sys.path.append(p)


_ensure_paths()

import concourse.bass as bass  # noqa: E402
import concourse.bacc as bacc  # noqa: E402
import concourse.mybir as mybir  # noqa: E402
import concourse.tile as tile  # noqa: E402
import concourse.hw_specs as hw_specs  # noqa: E402
from concourse.bass_utils import run_bass_kernel_spmd  # noqa: E402

B, S, D = 8, 4096, 1024
N_CORES = 8
ROWS = (B * S) // N_CORES  # 4096 rows per core
P = 128
NT = ROWS // P  # 32 row-tiles per core
G = 8  # tiles per closed-form group
THRESH = float(np.float32(0.99))
F32 = mybir.dt.float32
A = mybir.AluOpType
AF = mybir.ActivationFunctionType

_ACT_TABLE = "natural_log_exp_and_others"


def _patch_act_tables():
    """Force the act-table pass to use only the table holding Exp+Ln+Copy.

    The greedy pass otherwise bounces between an Exp-only and an Ln-only
    table (1.28us per load, 5 loads).  Positions must be preserved because
    the list index is the act_func_set_id walrus remaps.
    """
    if getattr(bacc, "_act_tables_patched", False):
        return

    def _filtered(arch):
        t = hw_specs.get_activation_tables(arch)
        return {n: (f if n == _ACT_TABLE else set()) for n, f in t.items()}

    bacc.get_activation_tables = _filtered
    bacc._act_tables_patched = True


def build_nc():
    _patch_act_tables()
    nc = bacc.Bacc("TRN2", target_bir_lowering=False, debug=False)

    x_ext = nc.declare_dram_parameter("x", [ROWS, D], F32, isOutput=False)
    w_ext = nc.declare_dram_parameter("W", [P, D], F32, isOutput=False)  # replicated
    b_ext = nc.declare_dram_parameter("nb", [P, 1], F32, isOutput=False)  # -b replicated
    st_ext = nc.declare_dram_parameter("out_state", [ROWS, D], F32, isOutput=True)
    pd_ext = nc.declare_dram_parameter("out_ponder", [ROWS, 1], F32, isOutput=True)

    # ponder viewed as (p, t): row 128*t + p  ->  partition p, column t
    pd_view = pd_ext[:, :].rearrange("(t p) one -> p (t one)", p=P)

    with tile.TileContext(nc) as tc:
        with (
            tc.tile_pool(name="const", bufs=1) as cpool,
            tc.tile_pool(name="xdata", bufs=NT) as xpool,
            tc.tile_pool(name="scratch", bufs=2) as spool,
            tc.tile_pool(name="small", bufs=4) as smp,
        ):
            # --- W and -b arrive pre-replicated across partitions ---
            wb = cpool.tile([P, D], F32)
            nc.sync.dma_start(wb[:], w_ext[:, :])
            nbcol = cpool.tile([P, 1], F32)
            nc.sync.dma_start(nbcol[:], b_ext[:, :])
            pond_all = cpool.tile([P, NT], F32)

            # pass 1: all loads + dot products, so no store ever blocks a load
            zbs = []
            xts_all = []
            for g in range(NT // G):
                zb = smp.tile([P, G], F32, tag="zb")
                zbs.append(zb)
            for t in range(NT):
                g, j = t // G, t % G
                xt = xpool.tile([P, D], F32, tag="xt")
                nc.sync.dma_start(xt[:], x_ext[bass.ts(t, P), :])
                xts_all.append(xt)
                scr = spool.tile([P, D], F32, tag="scr")
                nc.vector.affine_mul_reduce(
                    out=scr[:],
                    accum_out=zbs[g][:, j : j + 1],
                    in0=xt[:],
                    in1=wb[:],
                    scale=1.0,
                    bias=0.0,
                )

            # pass 2: per-group closed form + scale + store
            for g in range(NT // G):
                zb = zbs[g]
                xts = xts_all[g * G : (g + 1) * G]

                def st(tag):
                    return smp.tile([P, G], F32, tag=tag, name=tag)

                ez = st("ez")
                nc.scalar.activation(
                    ez[:], zb[:], AF.Exp, scale=-1.0, bias=nbcol[:]
                )  # e^-(z+b)
                den = st("den")
                nc.vector.tensor_scalar(den[:], ez[:], 1e37, 1.0, A.min, A.add)  # 1/p
                p_ = st("p")
                nc.vector.reciprocal(p_[:], den[:])
                om = st("om")
                nc.vector.tensor_scalar(om[:], p_[:], -1.0, 1.0, A.mult, A.add)  # 1-p
                om2 = st("om2")
                nc.vector.tensor_scalar(om2[:], om[:], 1e-38, None, A.max)
                lom = st("lom")
                nc.scalar.activation(lom[:], om2[:], AF.Ln)  # ln(1-p)
                q = st("q")
                nc.vector.tensor_scalar(q[:], den[:], THRESH, 30.0, A.mult, A.min)
                # e2 = min(floor(q), 20): cast-based floor with +-1 correction,
                # correct under any f32->i32 rounding mode (HW rounds, sim truncs)
                q2 = st("q2")
                nc.vector.tensor_scalar(q2[:], q[:], 0.5, None, A.subtract)
                qi = smp.tile([P, G], mybir.dt.int32, tag="qi", name="qi")
                nc.vector.tensor_copy(qi[:], q2[:])
                qf = st("qf")
                nc.vector.tensor_copy(qf[:], qi[:])
                c1 = st("c1")
                nc.vector.scalar_tensor_tensor(
                    c1[:], q[:], 1.0, qf[:], A.subtract, A.is_ge
                )  # [q-1 >= qf] = [q >= qf+1]
                c2 = st("c2")
                nc.vector.tensor_tensor(c2[:], q[:], qf[:], A.is_lt)
                ra = st("ra")
                nc.vector.tensor_tensor(ra[:], qf[:], c1[:], A.add)
                rb = st("rb")
                nc.vector.tensor_tensor(rb[:], ra[:], c2[:], A.subtract)
                e2 = st("e2")
                nc.vector.tensor_scalar(e2[:], rb[:], 20.0, None, A.min)
                h = st("h")
                nc.vector.tensor_scalar(h[:], e2[:], 19.5, None, A.is_lt)
                esp = st("esp")
                nc.vector.tensor_tensor(esp[:], e2[:], lom[:], A.mult)
                pw = st("pw")
                nc.scalar.activation(pw[:], esp[:], AF.Exp)  # (1-p)^e
                ep = st("ep")
                nc.vector.tensor_tensor(ep[:], e2[:], p_[:], A.mult)
                v = st("v")
                nc.vector.scalar_tensor_tensor(
                    v[:], ep[:], 1.0, h[:], A.subtract, A.mult
                )  # (ep-1)*h = u-h
                w2 = st("w2")
                nc.vector.tensor_tensor(w2[:], v[:], pw[:], A.mult)
                mm = st("mm")
                nc.vector.tensor_tensor(mm[:], pw[:], w2[:], A.add)
                mb = st("mb")
                nc.vector.tensor_scalar(mb[:], mm[:], -1.0, 1.0, A.mult, A.add)

                a2 = st("a2")
                nc.vector.tensor_tensor(a2[:], e2[:], ep[:], A.subtract)
                pp = st("pp")
                nc.vector.tensor_scalar(pp[:], p_[:], 18.0, None, A.add)
                b3 = st("b3")
                nc.vector.tensor_tensor(b3[:], pp[:], a2[:], A.subtract)
                c3 = st("c3")
                nc.vector.tensor_tensor(c3[:], h[:], b3[:], A.mult)
                nc.vector.tensor_scalar(
                    pond_all[:, g * G : (g + 1) * G], c3[:], -1.0, 20.0, A.mult, A.add
                )

                # --- scale x by m in place (alternate ACT/DVE), store out ---
                for j in range(G):
                    t = g * G + j
                    mcol = mb[:, j : j + 1]
                    if j % 2 == 0:
                        nc.scalar.activation(xts[j][:], xts[j][:], AF.Copy, scale=mcol)
                    else:
                        nc.vector.tensor_scalar(
                            xts[j][:], xts[j][:], mcol, None, A.mult
                        )
                    nc.sync.dma_start(st_ext[bass.ts(t, P), :], xts[j][:])


            nc.sync.dma_start(pd_view[:, :], pond_all[:])

    nc.compile()
    return nc


_NC_CACHE = []


def _get_nc():
    if not _NC_CACHE:
        _NC_CACHE.append(build_nc())
    return _NC_CACHE[0]


def make_in_maps(x, W, b):
    x = np.ascontiguousarray(x, dtype=np.float32)
    Wt = np.ascontiguousarray(
        np.broadcast_to(W.reshape(1, D), (P, D)), dtype=np.float32
    )
    nb = np.full((P, 1), -float(np.asarray(b).reshape(-1)[0]), dtype=np.float32)
    xs = x.reshape(N_CORES, ROWS, D)
    return [{"x": xs[c], "W": Wt, "nb": nb} for c in range(N_CORES)]


def kernel(x: np.ndarray, W: np.ndarray, b: np.ndarray):
    nc = _get_nc()
    in_maps = make_in_maps(x, W, b)
    res = run_bass_kernel_spmd(nc, in_maps, core_ids=list(range(N_CORES)))
    state = np.stack(
        [res.results[c]["out_state"] for c in range(N_CORES)], axis=0
    ).reshape(B, S, D)
    ponder = np.stack(
        [res.results[c]["out_ponder"] for c in range(N_CORES)], axis=0
    ).reshape(B, S, 1)
    return state, ponder


# revision 20
# speedup vs baseline: 1.4887x; 1.1254x over previous
"""Adaptive Computation Time kernel for 8 TRN2 NeuronCores.

Reference semantics: p = sigmoid(x @ W + b) is constant across the 20 ACT
steps, so the scan has a closed form per position:
  N_raw = floor(0.99/p) + 1          (first step n with n*p > 0.99)
  halted = N_raw <= 20
  e = min(floor(0.99/p), 20)          (exponent; = N-1 if halted else 20)
  state  = x * (1 - (1-p)^e * (halted ? e*p : 1))
  ponder = halted ? (e+1) + 1 - (e+1)*p : 20
with (1-p)^e = exp(e * ln(1-p)).

Sharding: data-parallel over batch (B=8 -> 1 batch element per core).

Engine plan per core (32 row-tiles of (128,1024)):
  sync seq   : 32 x-tile loads + const loads      (DMA, ~47us traffic)
  DVE        : 32 affine_mul_reduce (z=x.W) + half the scale-muls + reciprocal
  ACT        : Exp/Ln (one act table) + other half of scale-muls
  GpSimd     : closed-form elementwise chains on (128,G) z-groups
  tensor seq : 32 state-tile stores               (DMA, ~47us traffic)
"""

import os
import sys

import numpy as np


def _ensure_paths():
    for p in (
        "/root/.axon_site",
        "/root/.axon_site/_ro/trn_rl_repo",
        "/root/.axon_site/_ro/pypackages",
        "/opt/trn_rl_repo",
        "/opt/pypackages",
    ):
        if os.path.isdir(p) and p not in sys.path:
            sys.path.append(p)


_ensure_paths()

import concourse.bass as bass  # noqa: E402
import concourse.bacc as bacc  # noqa: E402
import concourse.mybir as mybir  # noqa: E402
import concourse.tile as tile  # noqa: E402
import concourse.hw_specs as hw_specs  # noqa: E402
from concourse.bass_utils import run_bass_kernel_spmd  # noqa: E402

B, S, D = 8, 4096, 1024
N_CORES = 8
ROWS = (B * S) // N_CORES  # 4096 rows per core
P = 128
NT = ROWS // P  # 32 row-tiles per core
G = 8  # tiles per closed-form group
THRESH = float(np.float32(0.99))
F32 = mybir.dt.float32
A = mybir.AluOpType
AF = mybir.ActivationFunctionType

_ACT_TABLE = "natural_log_exp_and_others"


def _patch_act_tables():
    """Force the act-table pass to use only the table holding Exp+Ln+Copy.

    The greedy pass otherwise bounces between an Exp-only and an Ln-only
    table (1.28us per load, 5 loads).  Positions must be preserved because
    the list index is the act_func_set_id walrus remaps.
    """
    if getattr(bacc, "_act_tables_patched", False):
        return

    def _filtered(arch):
        t = hw_specs.get_activation_tables(arch)
        return {n: (f if n == _ACT_TABLE else set()) for n, f in t.items()}

    bacc.get_activation_tables = _filtered
    bacc._act_tables_patched = True


def build_nc():
    _patch_act_tables()
    nc = bacc.Bacc("TRN2", target_bir_lowering=False, debug=False)

    x_ext = nc.declare_dram_parameter("x", [ROWS, D], F32, isOutput=False)
    w_ext = nc.declare_dram_parameter("W", [P, D], F32, isOutput=False)  # replicated
    b_ext = nc.declare_dram_parameter("nb", [P, 1], F32, isOutput=False)  # -b replicated
    st_ext = nc.declare_dram_parameter("out_state", [ROWS, D], F32, isOutput=True)
    pd_ext = nc.declare_dram_parameter("out_ponder", [ROWS, 1], F32, isOutput=True)

    # ponder viewed as (p, t): row 128*t + p  ->  partition p, column t
    pd_view = pd_ext[:, :].rearrange("(t p) one -> p (t one)", p=P)

    with tile.TileContext(nc) as tc:
        with (
            tc.tile_pool(name="const", bufs=1) as cpool,
            tc.tile_pool(name="xdata", bufs=NT) as xpool,
            tc.tile_pool(name="scratch", bufs=2) as spool,
            tc.tile_pool(name="small", bufs=4) as smp,
        ):
            # --- W and -b arrive pre-replicated across partitions ---
            wb = cpool.tile([P, D], F32)
            nc.sync.dma_start(wb[:], w_ext[:, :])
            nbcol = cpool.tile([P, 1], F32)
            nc.sync.dma_start(nbcol[:], b_ext[:, :])
            pond_all = cpool.tile([P, NT], F32)

            # pass 1: all loads + dot products, so no store ever blocks a load
            zbs = []
            xts_all = []
            for g in range(NT // G):
                zb = smp.tile([P, G], F32, tag="zb")
                zbs.append(zb)
            for t in range(NT):
                g, j = t // G, t % G
                xt = xpool.tile([P, D], F32, tag="xt")
                nc.sync.dma_start(xt[:], x_ext[bass.ts(t, P), :])
                xts_all.append(xt)
                scr = spool.tile([P, D], F32, tag="scr")
                nc.vector.affine_mul_reduce(
                    out=scr[:],
                    accum_out=zbs[g][:, j : j + 1],
                    in0=xt[:],
                    in1=wb[:],
                    scale=1.0,
                    bias=0.0,
                )

            # pass 2: per-group closed form + scale + store
            for g in range(NT // G):
                zb = zbs[g]
                xts = xts_all[g * G : (g + 1) * G]

                def st(tag):
                    return smp.tile([P, G], F32, tag=tag, name=tag)

                ez = st("ez")
                nc.scalar.activation(
                    ez[:], zb[:], AF.Exp, scale=-1.0, bias=nbcol[:]
                )  # e^-(z+b)
                den = st("den")
                nc.vector.tensor_scalar(den[:], ez[:], 1e37, 1.0, A.min, A.add)  # 1/p
                p_ = st("p")
                nc.vector.reciprocal(p_[:], den[:])
                om = st("om")
                nc.vector.tensor_scalar(om[:], p_[:], -1.0, 1.0, A.mult, A.add)  # 1-p
                om2 = st("om2")
                nc.vector.tensor_scalar(om2[:], om[:], 1e-38, None, A.max)
                lom = st("lom")
                nc.scalar.activation(lom[:], om2[:], AF.Ln)  # ln(1-p)
                q = st("q")
                nc.vector.tensor_scalar(q[:], den[:], THRESH, 30.0, A.mult, A.min)
                # e2 = min(floor(q), 20): cast-based floor with +-1 correction,
                # correct under any f32->i32 rounding mode (HW rounds, sim truncs)
                q2 = st("q2")
                nc.vector.tensor_scalar(q2[:], q[:], 0.5, None, A.subtract)
                qi = smp.tile([P, G], mybir.dt.int32, tag="qi", name="qi")
                nc.vector.tensor_copy(qi[:], q2[:])
                qf = st("qf")
                nc.vector.tensor_copy(qf[:], qi[:])
                c1 = st("c1")
                nc.vector.scalar_tensor_tensor(
                    c1[:], q[:], 1.0, qf[:], A.subtract, A.is_ge
                )  # [q-1 >= qf] = [q >= qf+1]
                c2 = st("c2")
                nc.vector.tensor_tensor(c2[:], q[:], qf[:], A.is_lt)
                ra = st("ra")
                nc.vector.tensor_tensor(ra[:], qf[:], c1[:], A.add)
                rb = st("rb")
                nc.vector.tensor_tensor(rb[:], ra[:], c2[:], A.subtract)
                e2 = st("e2")
                nc.vector.tensor_scalar(e2[:], rb[:], 20.0, None, A.min)
                h = st("h")
                nc.vector.tensor_scalar(h[:], e2[:], 19.5, None, A.is_lt)
                esp = st("esp")
                nc.vector.tensor_tensor(esp[:], e2[:], lom[:], A.mult)
                pw = st("pw")
                nc.scalar.activation(pw[:], esp[:], AF.Exp)  # (1-p)^e
                ep = st("ep")
                nc.vector.tensor_tensor(ep[:], e2[:], p_[:], A.mult)
                v = st("v")
                nc.vector.scalar_tensor_tensor(
                    v[:], ep[:], 1.0, h[:], A.subtract, A.mult
                )  # (ep-1)*h = u-h
                w2 = st("w2")
                nc.vector.tensor_tensor(w2[:], v[:], pw[:], A.mult)
                mm = st("mm")
                nc.vector.tensor_tensor(mm[:], pw[:], w2[:], A.add)
                mb = st("mb")
                nc.vector.tensor_scalar(mb[:], mm[:], -1.0, 1.0, A.mult, A.add)

                a2 = st("a2")
                nc.vector.tensor_tensor(a2[:], e2[:], ep[:], A.subtract)
                pp = st("pp")
                nc.vector.tensor_scalar(pp[:], p_[:], 18.0, None, A.add)
                b3 = st("b3")
                nc.vector.tensor_tensor(b3[:], pp[:], a2[:], A.subtract)
                c3 = st("c3")
                nc.vector.tensor_tensor(c3[:], h[:], b3[:], A.mult)
                nc.vector.tensor_scalar(
                    pond_all[:, g * G : (g + 1) * G], c3[:], -1.0, 20.0, A.mult, A.add
                )

                # --- scale x by m in place (alternate ACT/DVE), store out ---
                for j in range(G):
                    t = g * G + j
                    mcol = mb[:, j : j + 1]
                    if j % 2 == 0:
                        nc.scalar.activation(xts[j][:], xts[j][:], AF.Copy, scale=mcol)
                        nc.scalar.dma_start(st_ext[bass.ts(t, P), :], xts[j][:])
                    else:
                        nc.vector.tensor_scalar(
                            xts[j][:], xts[j][:], mcol, None, A.mult
                        )
                        nc.sync.dma_start(st_ext[bass.ts(t, P), :], xts[j][:])


            nc.sync.dma_start(pd_view[:, :], pond_all[:])

    nc.compile()
    return nc


_NC_CACHE = []


def _get_nc():
    if not _NC_CACHE:
        _NC_CACHE.append(build_nc())
    return _NC_CACHE[0]


def make_in_maps(x, W, b):
    x = np.ascontiguousarray(x, dtype=np.float32)
    Wt = np.ascontiguousarray(
        np.broadcast_to(W.reshape(1, D), (P, D)), dtype=np.float32
    )
    nb = np.full((P, 1), -float(np.asarray(b).reshape(-1)[0]), dtype=np.float32)
    xs = x.reshape(N_CORES, ROWS, D)
    return [{"x": xs[c], "W": Wt, "nb": nb} for c in range(N_CORES)]


def kernel(x: np.ndarray, W: np.ndarray, b: np.ndarray):
    nc = _get_nc()
    in_maps = make_in_maps(x, W, b)
    res = run_bass_kernel_spmd(nc, in_maps, core_ids=list(range(N_CORES)))
    state = np.stack(
        [res.results[c]["out_state"] for c in range(N_CORES)], axis=0
    ).reshape(B, S, D)
    ponder = np.stack(
        [res.results[c]["out_ponder"] for c in range(N_CORES)], axis=0
    ).reshape(B, S, 1)
    return state, ponder
